# revision 23
# baseline (speedup 1.0000x reference)
"""Trainium2 Bass kernel for CGC3 MoE routing (nn_CGC3_48163763257633).

Full-input contract: kernel(**inputs) takes the unsharded inputs and returns
the full [B, T+1, O] output. Internally: data-parallel over batch across 8
NeuronCores (weights replicated), no collectives.

Per-core program (B_local=1024), v2 (single-shot latency tuned):
  - all matmul operands bf16 (same 1 cyc/row PE rate as f32r, half the DMA
    bytes and half the LDWEIGHTS cost; whole-kernel rel err ~4e-3)
  - head: xT k-tiles DMA'd split across the SP and ACT hardware DGE queues,
    first expert's W1 on the otherwise-idle Pool SWDGE queue; the gate
    contraction is split kt 0-3 / kt 4-7 into two PSUM tiles (multi-region,
    consecutive accumulation groups per region) so gate matmuls track xT
    arrival — this also burns the PE pstate ramp inside the DMA window;
    softmax skips the max-subtraction (logits bounded ~|8|) so ACT stays
    clear for the first layer-1 PSUM drains
  - layer 1 computed transposed: hT[e] = relu(W1[e].T-slices @ xT) -> [H, B]
    bf16 so layer 2 (contraction over H) needs no on-chip transposes
  - layer 2: eo[e] = relu(hT[e].T @ W2[e]) -> [B, O] f32, drained from PSUM
    by ACT while DVE applies the 2-3 gate scalars per expert into the 3
    output-group accumulators (scalar_tensor_tensor FMA)
  - expert order 4,5,6,7,0,1,2,3: the task-0 group's accumulators complete
    two experts early so their writeback DMA hides under remaining compute;
    the final 16 output tiles alternate SP/ACT queues during expert 3's L2
"""

import sys

sys.path.insert(0, "/opt/trn_rl_repo")

import numpy as np

import concourse.bass as bass  # noqa: F401  (AP helpers)
import concourse.mybir as mybir
import concourse.tile as tile
from concourse import bacc
from concourse.bass_utils import run_bass_kernel_spmd

# ---- problem constants (hardcoded per contract) ----
B, D = 8192, 1024
H, O = 1024, 512
T, E_T, E_S = 2, 2, 4
E = T * E_T + E_S          # 8 experts
NG = T + 1                 # 3 output groups (task0, task1, shared)
GC = T * (E_T + E_S) + E   # 20 gate columns: 6 + 6 + 8
NCORES = 8
BL = B // NCORES           # 1024 rows per core
P = 128
KT = D // P                # 8 k-tiles over D
HT = H // P                # 8 partition-tiles over H
BT = BL // P               # 8 batch tiles per core
BH = BL // 512             # 2 batch halves (layer-1 free dim)

# expert processing order: shared experts first, then task experts with
# task-0's pair early so group-0 accumulators finish 2 experts before the end
EXPERT_ORDER = [4, 5, 6, 7, 0, 1, 2, 3]

F32 = mybir.dt.float32
BF16 = mybir.dt.bfloat16
RELU = mybir.ActivationFunctionType.Relu
EXP = mybir.ActivationFunctionType.Exp
MAX = mybir.AluOpType.max
MULT = mybir.AluOpType.mult
ADD = mybir.AluOpType.add
AXX = mybir.AxisListType.X


def _expert_contribs():
    """(group, gate-column) pairs each expert feeds.

    Task i's softmax columns order its own E_T experts first, then the E_S
    shared experts. The shared tower (group NG-1) covers all E experts.
    """
    contribs = {e: [] for e in range(E)}
    for i in range(T):
        base = i * (E_T + E_S)
        for j, e in enumerate(
            list(range(i * E_T, (i + 1) * E_T)) + list(range(E - E_S, E))
        ):
            contribs[e].append((i, base + j))
    shared_base = T * (E_T + E_S)
    for e in range(E):
        contribs[e].append((T, shared_base + e))
    return contribs


def _last_contributor():
    """group -> expert (in EXPERT_ORDER) whose drain completes the group."""
    contribs = _expert_contribs()
    last = {}
    for pos, e in enumerate(EXPERT_ORDER):
        for (grp, _col) in contribs[e]:
            last[grp] = e
    return last


def _build_program(has_b2: bool, repeat: int = 1):
    """repeat>1 replicates the whole compute body inside one NEFF (timing
    harness only — output is then wrong; slope over repeat isolates per-run
    exec time from the large fixed axon dispatch overhead)."""
    WDT = ADT = BF16
    nc = bacc.Bacc("TRN2", target_bir_lowering=False, debug=False,
                   num_devices=NCORES)

    xT_d = nc.dram_tensor("xT", [D, BL], ADT, kind="ExternalInput").ap()
    w1_d = nc.dram_tensor("W1", [E, D, H], WDT, kind="ExternalInput").ap()
    w2_d = nc.dram_tensor("W2", [E, H, O], WDT, kind="ExternalInput").ap()
    wcat_d = nc.dram_tensor("Wcat", [D, GC], WDT, kind="ExternalInput").ap()
    b1_d = nc.dram_tensor("b1r", [P, E * HT], F32, kind="ExternalInput").ap()
    if has_b2:
        b2_d = nc.dram_tensor("b2", [E, O], F32, kind="ExternalInput").ap()
    out_d = nc.dram_tensor("out", [BL, NG, O], F32, kind="ExternalOutput").ap()

    contribs = _expert_contribs()
    last_contrib = _last_contributor()

    with tile.TileContext(nc) as tc:
        with (
            tc.tile_pool(name="xt", bufs=KT) as xt_pool,
            tc.tile_pool(name="w1", bufs=16) as w1_pool,
            tc.tile_pool(name="w2", bufs=16) as w2_pool,
            tc.tile_pool(name="ht", bufs=10) as ht_pool,
            tc.tile_pool(name="eo", bufs=4) as eo_pool,
            tc.tile_pool(name="acc", bufs=NG * BT) as acc_pool,
            tc.tile_pool(name="gate", bufs=BT) as gate_pool,
            tc.tile_pool(name="small", bufs=1) as small_pool,
            tc.tile_pool(name="b2p", bufs=2) as b2_pool,
            tc.tile_pool(name="tmp", bufs=6) as tmp_pool,
            tc.tile_pool(name="psmm", bufs=6, space="PSUM") as psmm_pool,
            tc.tile_pool(name="psg", bufs=1, space="PSUM") as psg_pool,
        ):
            # ---- persistent loads ----
            # wcat on ACT first (gates need it before the first xT chunk);
            # xT as [P, 512] half-tiles alternating SP/ACT queues, first
            # halves of every k-tile before second halves so the first gate
            # matmuls (bt 0-3) start as early as possible
            wcat_t = small_pool.tile([P, KT, GC], WDT)
            nc.scalar.dma_start(
                out=wcat_t, in_=wcat_d.rearrange("(k p) g -> p k g", p=P)
            )
            b1t = small_pool.tile([P, E * HT], F32)
            nc.sync.dma_start(out=b1t, in_=b1_d)

            xts = []
            for kt in range(KT):
                t = xt_pool.tile([P, BL], ADT)
                eng = nc.sync if kt % 2 == 0 else nc.scalar
                eng.dma_start(out=t, in_=xT_d[kt * P:(kt + 1) * P, :])
                xts.append(t)

            for _rep in range(repeat):
                # ---- gates, split contraction: kt 0-3 accumulate while
                # kt 4-7 still stream in, then kt 4-7, then a DVE add joins
                # the halves. Each [*, bt, :] region's accumulation group is
                # consecutive (required); regions share one PSUM tile. ----
                gpa = psg_pool.tile([P, BT, GC], F32)
                gpb = psg_pool.tile([P, BT, GC], F32)
                KH = KT // 2
                for bt in range(BT):
                    for kt in range(KH):
                        nc.tensor.matmul(
                            gpa[:, bt, :],
                            xts[kt][:, bt * P:(bt + 1) * P],
                            wcat_t[:, kt, :],
                            start=(kt == 0),
                            stop=(kt == KH - 1),
                            skip_group_check=True,
                        )
                for bt in range(BT):
                    for kt in range(KH, KT):
                        nc.tensor.matmul(
                            gpb[:, bt, :],
                            xts[kt][:, bt * P:(bt + 1) * P],
                            wcat_t[:, kt, :],
                            start=(kt == KH),
                            stop=(kt == KT - 1),
                            skip_group_check=True,
                        )
                # DVE may read only one PSUM operand per instruction: stage
                # one half through SBUF, then add
                gbs = gate_pool.tile([P, BT, GC], F32, tag="gbs")
                nc.vector.tensor_copy(out=gbs, in_=gpb)
                gl = gate_pool.tile([P, BT, GC], F32, tag="gl")
                nc.vector.tensor_tensor(out=gl, in0=gpa, in1=gbs, op=ADD)
                # softmax without max-subtraction (logits are bounded ~|8|,
                # exp is safe in f32): one ACT exp per bt over all 20 cols,
                # per-group sum/recip/scale on DVE — keeps ACT free for the
                # layer-1 relu drains that gate PSUM reuse
                gts = []
                for bt in range(BT):
                    ge = gate_pool.tile([P, GC], F32, tag="ge")
                    nc.scalar.activation(
                        out=ge, in_=gl[:, bt, :], func=EXP, scale=1.0,
                    )
                    gt = gate_pool.tile([P, GC], F32)
                    for gi in range(T + 1):
                        c0 = gi * (E_T + E_S) if gi < T else T * (E_T + E_S)
                        c1 = c0 + (E_T + E_S if gi < T else E)
                        es = tmp_pool.tile([P, 1], F32)
                        nc.vector.tensor_reduce(
                            out=es, in_=ge[:, c0:c1], axis=AXX,
                            op=ADD,
                        )
                        rs = tmp_pool.tile([P, 1], F32)
                        nc.vector.reciprocal(out=rs, in_=es)
                        nc.vector.tensor_scalar_mul(gt[:, c0:c1], ge[:, c0:c1], rs)
                    gts.append(gt)

                # ---- experts ----
                acc_tiles = {}
                out_dma_flip = 0
                for pos, e in enumerate(EXPERT_ORDER):
                    b2bt = None
                    if has_b2:
                        # b2[e] broadcast across partitions (DMA stride-0)
                        b2bt = b2_pool.tile([P, O], F32, tag="b2")
                        b2_row = b2_d[e, :]
                        b2_bcast = bass.AP(
                            tensor=b2_row.tensor,
                            offset=b2_row.offset,
                            ap=[[0, P]] + [list(a) for a in b2_row.ap],
                        )
                        nc.sync.dma_start(out=b2bt, in_=b2_bcast)
                    w1ts = []
                    for kt in range(KT):
                        t = w1_pool.tile([P, H], WDT)
                        # W1 alternates SP and the otherwise-idle Pool SWDGE
                        # queue (first expert leans harder on Pool so layer-1
                        # starts early; ACT stays free for PSUM relu drains)
                        if pos == 0 and _rep == 0:
                            eng = nc.sync if kt % 3 == 2 else nc.gpsimd
                        else:
                            eng = nc.sync if kt % 2 == 0 else nc.gpsimd
                        eng.dma_start(out=t, in_=w1_d[e, kt * P:(kt + 1) * P, :])
                        w1ts.append(t)
                    w2ts = []
                    for ht in range(HT):
                        t = w2_pool.tile([P, O], WDT)
                        nc.sync.dma_start(out=t, in_=w2_d[e, ht * P:(ht + 1) * P, :])
                        w2ts.append(t)

                    # layer 1: hT[e] = relu(x @ W1[e] (+ b1))^T -> [H, BL] bf16
                    # kt-inner so at most ~3 ht psum pairs are live at once
                    hts = []
                    for ht in range(HT):
                        htile = ht_pool.tile([P, BL], ADT)
                        hpss = []
                        for bh in range(BH):
                            hps = psmm_pool.tile([P, 512], F32, tag="mm")
                            hpss.append(hps)
                        for kt in range(KT):
                            for bh in range(BH):
                                nc.tensor.matmul(
                                    hpss[bh],
                                    w1ts[kt][:, ht * P:(ht + 1) * P],
                                    xts[kt][:, bh * 512:(bh + 1) * 512],
                                    start=(kt == 0),
                                    stop=(kt == KT - 1),
                                )
                        for bh in range(BH):
                            nc.scalar.activation(
                                out=htile[:, bh * 512:(bh + 1) * 512], in_=hpss[bh],
                                func=RELU, bias=b1t[:, e * HT + ht:e * HT + ht + 1],
                                scale=1.0,
                            )
                        hts.append(htile)

                    # layer 2 + fused gating drain (+ writeback when final)
                    for bt in range(BT):
                        ops = psmm_pool.tile([P, O], F32, tag="mm")
                        for ht in range(HT):
                            nc.tensor.matmul(
                                ops,
                                hts[ht][:, bt * P:(bt + 1) * P],
                                w2ts[ht],
                                start=(ht == 0),
                                stop=(ht == HT - 1),
                            )
                        last_expert = pos == len(EXPERT_ORDER) - 1
                        eot = eo_pool.tile([P, O], F32, tag="eot")
                        if has_b2:
                            pre = eo_pool.tile([P, O], F32, tag="eot")
                            nc.vector.scalar_tensor_tensor(
                                out=pre, in0=ops, scalar=1.0, in1=b2bt,
                                op0=MULT, op1=ADD,
                            )
                            nc.scalar.activation(out=eot, in_=pre, func=RELU)
                        else:
                            nc.scalar.activation(out=eot, in_=ops, func=RELU)

                        for (grp, col) in contribs[e]:
                            key = (grp, bt)
                            sc = gts[bt][:, col:col + 1]
                            if key not in acc_tiles:
                                at = acc_pool.tile([P, O], F32)
                                acc_tiles[key] = at
                                nc.vector.tensor_scalar_mul(at, eot, sc)
                            else:
                                nc.vector.scalar_tensor_tensor(
                                    out=acc_tiles[key], in0=eot, scalar=sc,
                                    in1=acc_tiles[key], op0=MULT, op1=ADD,
                                )
                            # writeback as soon as the group's last expert has
                            # gated this batch tile in. During the last expert
                            # keep ACT's queue clear for the relu drains: SP
                            # carries everything except the final tile's pair,
                            # which goes ACT+SP in parallel.
                            if last_contrib[grp] == e:
                                if last_expert:
                                    eng = (nc.scalar
                                           if bt == BT - 1 and grp == NG - 1
                                           else nc.sync)
                                else:
                                    eng = (nc.sync if out_dma_flip % 2 == 0
                                           else nc.scalar)
                                    out_dma_flip += 1
                                eng.dma_start(
                                    out=out_d[bt * P:(bt + 1) * P, grp, :],
                                    in_=acc_tiles[key],
                                )

    nc.compile()
    return nc


_CACHE = {}


def _get_program(has_b2: bool):
    return _get_program_rep(has_b2, 1)


def _get_program_rep(has_b2: bool, repeat: int):
    key = (has_b2, repeat)
    if key not in _CACHE:
        _CACHE[key] = _build_program(has_b2, repeat)
    return _CACHE[key]


def make_in_maps(x, W1, b1, W2, b2, Wg, Wgs):
    """Host-side shard/layout prep -> per-core input dicts."""
    import ml_dtypes
    bf = ml_dtypes.bfloat16
    x = np.ascontiguousarray(x, dtype=np.float32)
    b1 = np.asarray(b1, dtype=np.float32)
    b2 = np.asarray(b2, dtype=np.float32)
    Wcat = np.concatenate(
        [Wg[i] for i in range(T)] + [Wgs], axis=1
    ).astype(np.float32)  # [D, 20]
    b1r = np.ascontiguousarray(
        b1.reshape(E, HT, P).transpose(2, 0, 1).reshape(P, E * HT)
    )
    has_b2 = bool(np.any(b2))
    W1c = np.ascontiguousarray(np.asarray(W1, dtype=np.float32)).astype(bf)
    W2c = np.ascontiguousarray(np.asarray(W2, dtype=np.float32)).astype(bf)
    Wcatc = Wcat.astype(bf)
    in_maps = []
    for c in range(NCORES):
        xs = x[c * BL:(c + 1) * BL]
        m = {
            "xT": np.ascontiguousarray(xs.T).astype(bf),
            "W1": W1c,
            "W2": W2c,
            "Wcat": Wcatc,
            "b1r": b1r,
        }
        if has_b2:
            m["b2"] = b2
        in_maps.append(m)
    return in_maps, has_b2


def kernel(x, W1, b1, W2, b2, Wg, Wgs):
    in_maps, has_b2 = make_in_maps(x, W1, b1, W2, b2, Wg, Wgs)
    nc = _get_program(has_b2)
    res = run_bass_kernel_spmd(nc, in_maps, list(range(NCORES)))
    return np.concatenate([r["out"] for r in res.results], axis=0)
